# revision 31
# baseline (speedup 1.0000x reference)
"""NT-Xent loss kernel for Trainium2, 8 NeuronCores.

Problem: B=4096 per view, D=128, temperature=0.1.
reps = concat([zjs, zis]) -> [8192, 128]; normalize rows; sim = normed @ normed.T;
loss = mean_i(-pos_i/T + logsumexp_{j!=i}(sim_ij/T)).

Symmetric-shard strategy (static SPMD, no collectives; host assembles):
  sim is symmetric, so each unordered pair needs computing once.  Core c gets
  reps rotated by -1024c rows (host-packed [p, t, d] bf16, 40 tiles = rows
  0..5119 of its rotated frame).  It computes the exp-block for its 1024 rows
  x 5120 cols (local column blocks 0..4 of 8):
    - blocks 0 (self-diagonal) and 4 (antipodal, recomputed by the partner
      core) contribute row sums only (ACT exp in place on PSUM / DVE
      Schraudolph-bf16 bit-trick exp, both with fused row accumulation),
    - blocks 1-3 are written as fp8e4 tiles; their row sums feed the local
      rows AND their column sums (fp8 DoubleRow ones-matmul over row pairs)
      are the mirror contributions to rows owned by cores c+1..c+3.  Cols
      5120..8191 of this core's rows arrive symmetrically as colsums from
      cores c-1..c-3.
  Everything on-device is in exp(10*s - 2) units (keeps fp8 in range; diag
  stays f32).  Outputs per core: own-row sums (diag-subtracted), positive-pair
  dots, 6x512 column sums.  The host assembles S per row across cores and
  takes the final ln in float64 (no on-device logs at all).

Scheduling notes: startup loads are split 2 tiles x 4 queues so the first
row-chunk's stats/transposes begin ~3us earlier; PE warm-up matmuls run
during the load phase to climb out of the low DVFS p-state; chunk order is
A | B | C/D interleaved per row-chunk (D's Schraudolph exps on DVE dovetail
with C's table exps on ACT); mirror colsums run after the next chunk's sims
so the PE never waits on an ACT->EXPB dependency.
"""

import numpy as np

B = 4096
D = 128
TWO_B = 2 * B
P = 128
NT = 40                   # loaded row tiles per core (rows 0..5119 local)
NLOCAL = NT * P           # 5120
MI = 8                    # own row-chunks (128 rows each -> 1024 rows)
NCORES = 8
ROWS_PER_CORE = TWO_B // NCORES  # 1024
INV_T = 10.0
SHIFT = 2.0               # on-device unit: exp(10 s - SHIFT)
LOG2E = 1.4426950408889634
SCHRAUDOLPH_C = 7.37      # calibrated: zero-mean bf16 bit-trick exp
A_DVE = float(np.float32(INV_T * 128.0 * LOG2E))
B_DVE = float(np.float32(128.0 * (127.0 - SHIFT * LOG2E) - SCHRAUDOLPH_C))

_CACHE = {}


def _dedup_ldweights(nc, mybir):
    """Drop InstLdweights that reload the identical stationary AP with no
    waits/updates: the PE array still holds those weights (bass emits one
    load per matmul; consecutive same-weight matmuls don't need it)."""
    removed = 0
    for fn in nc.m.functions:
        for blk in fn.blocks:
            insts = list(blk.instructions)
            new = []
            last_sig = None
            changed = False
            for i in insts:
                if isinstance(i, mybir.InstLdweights):
                    sig = str(i.ins[0]) + "|" + str(i.perf_mode)
                    if sig == last_sig and not i.has_wait() and not i.has_update():
                        removed += 1
                        changed = True
                        continue
                    last_sig = sig
                new.append(i)
            if changed:
                blk.instructions = new
    return removed


def build_nc():
    import concourse.bacc as bacc
    import concourse.bass as bass
    import concourse.mybir as mybir
    import concourse.tile as tile

    f32 = mybir.dt.float32
    bf16 = mybir.dt.bfloat16
    i16 = mybir.dt.int16
    fp8 = mybir.dt.float8e4
    AX = mybir.AxisListType
    OP = mybir.AluOpType
    AF = mybir.ActivationFunctionType
    DR = mybir.MatmulPerfMode.DoubleRow

    # Keep Ln+Exp in one activation table set (avoid mid-kernel reloads).
    from concourse import hw_specs

    _orig_tables = hw_specs.get_activation_tables

    def _patched_tables(arch):
        t = {k: set(v) for k, v in _orig_tables(arch).items()}
        for name, s in t.items():
            if name != "natural_log_exp_and_others":
                s.discard(AF.Exp)
                s.discard(AF.Ln)
        return t

    bacc.get_activation_tables = _patched_tables

    nc = bacc.Bacc(
        "TRN2",
        target_bir_lowering=False,
        debug=False,
        num_devices=NCORES,
    )
    # host-tiled: reps_h[p, t*128 + d] = bf16(reps_rot[t*128 + p, d]), t<40
    reps_h = nc.declare_dram_parameter("reps", [P, NLOCAL], bf16, isOutput=False)
    ones_h = nc.declare_dram_parameter("ones8", [P, 2, 32], fp8, isOutput=False)
    sp_h = nc.declare_dram_parameter("sownpos", [P, 2 * MI], f32, isOutput=True)
    cs_h = nc.declare_dram_parameter("cs", [1, 3072], f32, isOutput=True)

    with tile.TileContext(nc) as tc:
        with (
            tc.tile_pool(name="persist", bufs=1) as persist,
            tc.tile_pool(name="psum", bufs=2, space="PSUM") as psum,
            tc.tile_pool(name="scratch", bufs=2) as scratch,
        ):
            bias_m2 = persist.tile([P, 1], f32)
            nc.vector.memset(bias_m2, -SHIFT)
            ONES = persist.tile([P, 2, 32], fp8)
            nc.sync.dma_start(out=ONES, in_=ones_h[:, :, :])
            RAW = persist.tile([P, NT, P], bf16)
            SQ = persist.tile([P, NT, P], f32)
            HI = persist.tile([P, NT, P], bf16)
            HIT = persist.tile([P, NLOCAL], bf16)
            SS = persist.tile([P, NT], f32)
            LNSS = persist.tile([P, NT], f32)
            SCL = persist.tile([P, NT], f32)
            SPART = persist.tile([P, MI, 4], f32)
            SPARTV = persist.tile([P, MI, 2], f32)
            EXPB = persist.tile([P, MI, 3072], fp8)
            CS = persist.tile([1, 3072], f32)
            DIAG = persist.tile([P, MI], f32)
            POS = persist.tile([P, MI], f32)
            nc.vector.memset(SPART, 0.0)
            nc.vector.memset(SPARTV, 0.0)

            reps_t = reps_h[:, :].rearrange("p (t d) -> p t d", d=P)
            HIT3 = HIT.rearrange("d (t p) -> d t p", p=P)

            def scl_bcast(a, b):
                s = SCL[:, a:b]
                return bass.AP(
                    tensor=s.tensor, offset=s.offset, ap=list(s.ap) + [[0, P]]
                )

            def slice_stats(x, y):
                nc.vector.tensor_mul(SQ[:, x:y, :], RAW[:, x:y, :], RAW[:, x:y, :])
                nc.vector.reduce_sum(out=SS[:, x:y], in_=SQ[:, x:y, :], axis=AX.X)
                nc.scalar.activation(out=LNSS[:, x:y], in_=SS[:, x:y], func=AF.Ln)
                nc.scalar.activation(out=SCL[:, x:y], in_=LNSS[:, x:y],
                                     func=AF.Exp, scale=-0.5)

            def group0():
                # tiles 0..7 gate everything.  All stats pieces emit before
                # any HI-mul: an interleaved order serializes the pieces
                # through the DVE<->ACT rsqrt ping-pong (in-order queues).
                pieces = [(0, 3, nc.gpsimd), (3, 6, nc.sync), (6, 8, nc.scalar)]
                for x, y, ld in pieces:
                    ld.dma_start(out=RAW[:, x:y, :], in_=reps_t[:, x:y, :])
                for x, y, _ in pieces:
                    slice_stats(x, y)
                for (x, y, _), tr in zip(pieces, [nc.sync, nc.scalar, nc.sync]):
                    nc.vector.tensor_mul(
                        HI[:, x:y, :], RAW[:, x:y, :], scl_bcast(x, y)
                    )
                    tr.dma_start_transpose(
                        out=HIT3[:, x:y, :], in_=HI[:, x:y, :]
                    )

            def group(a, b, sq_eng=None):
                mid = a + (b - a) // 2
                splits = [(a, mid, nc.gpsimd), (mid, b, nc.scalar)]
                for x, y, eng in splits:
                    eng.dma_start(out=RAW[:, x:y, :], in_=reps_t[:, x:y, :])
                sq = sq_eng.tensor_mul if sq_eng else nc.vector.tensor_mul
                for x, y, _ in splits:
                    sq(SQ[:, x:y, :], RAW[:, x:y, :], RAW[:, x:y, :])
                    nc.vector.reduce_sum(
                        out=SS[:, x:y], in_=SQ[:, x:y, :], axis=AX.X
                    )
                nc.scalar.activation(out=LNSS[:, a:b], in_=SS[:, a:b], func=AF.Ln)
                nc.scalar.activation(
                    out=SCL[:, a:b], in_=LNSS[:, a:b], func=AF.Exp, scale=-0.5
                )
                for x in range(a, b, 4):
                    nc.vector.tensor_mul(
                        HI[:, x : x + 4, :],
                        RAW[:, x : x + 4, :],
                        scl_bcast(x, x + 4),
                    )
                    nc.sync.dma_start_transpose(
                        out=HIT3[:, x : x + 4, :], in_=HI[:, x : x + 4, :]
                    )

            group0()
            # diag_i = ||hi_i||^2 with the same bf16 values the matmul uses
            for mi in range(MI):
                jd = scratch.tile([P, P], f32, tag="ttr_junk")
                nc.vector.scalar_tensor_tensor(
                    out=jd,
                    in0=HI[:, mi, :],
                    scalar=1.0,
                    in1=HI[:, mi, :],
                    op0=OP.mult,
                    op1=OP.mult,
                    accum_out=DIAG[:, mi : mi + 1],
                )
            group(8, 20)
            group(20, 32, sq_eng=nc.gpsimd)
            group(32, 40, sq_eng=nc.gpsimd)
            # positive-pair dots: rows mi pair with antipodal tile 32+mi
            for mi in range(MI):
                jp = scratch.tile([P, P], f32, tag="ttr_junk")
                nc.vector.scalar_tensor_tensor(
                    out=jp,
                    in0=HI[:, mi, :],
                    scalar=1.0,
                    in1=HI[:, NT - MI + mi, :],
                    op0=OP.mult,
                    op1=OP.mult,
                    accum_out=POS[:, mi : mi + 1],
                )

            # ---------------- Phase B ----------------------------------------
            def sims(mi, c0, ncols):
                lhsT = HIT[:, mi * P : (mi + 1) * P]
                pg = psum.tile([P, 1536], f32, tag="pg")
                for k in range(ncols // 512):
                    nc.tensor.matmul(
                        pg[:, k * 512 : (k + 1) * 512],
                        lhsT,
                        HIT[:, c0 + k * 512 : c0 + (k + 1) * 512],
                        start=True,
                        stop=True,
                    )
                return pg

            def exp_act(pg, mi, ci, ncols, fp8_off=None):
                out = (
                    pg[:, :ncols]
                    if fp8_off is None
                    else EXPB[:, mi, fp8_off : fp8_off + ncols]
                )
                nc.scalar.activation(
                    out=out,
                    in_=pg[:, :ncols],
                    func=AF.Exp,
                    scale=INV_T,
                    bias=bias_m2,
                    accum_out=SPART[:, mi, ci : ci + 1],
                )

            def exp_dve_half(pgd, mi, h):
                jq = scratch.tile([P, 512], i16, tag="jq")
                nc.vector.tensor_scalar(
                    out=jq,
                    in0=pgd[:, :],
                    scalar1=A_DVE,
                    scalar2=B_DVE,
                    op0=OP.mult,
                    op1=OP.add,
                )
                nc.vector.reduce_sum(
                    out=SPARTV[:, mi, h : h + 1], in_=jq.bitcast(bf16), axis=AX.X
                )

            def sims_d_dve(mi):
                # block 4 on its own 1-bank psum ring: decouples the DVE
                # consumer from ACT's ring so the PE never waits on both
                lhsT = HIT[:, mi * P : (mi + 1) * P]
                for h in range(2):
                    pgd = psum.tile([P, 512], f32, tag="cs")
                    nc.tensor.matmul(
                        pgd[:, :],
                        lhsT,
                        HIT[:, 4096 + h * 512 : 4096 + (h + 1) * 512],
                        start=True,
                        stop=True,
                    )
                    exp_dve_half(pgd, mi, h)

            def colsum(groups, copy_eng):
                # mirror colsums over this core's 1024 rows: fp8 DoubleRow
                # ones-matmul, 2 row-tiles per pass, accumulate 4 passes
                for g in groups:
                    pc = psum.tile([P, 512], f32, tag="cs")
                    for k in range(4):
                        nc.tensor.matmul(
                            pc[0:32, :],
                            ONES[:, :, :],
                            EXPB[:, 2 * k : 2 * k + 2, g * 512 : (g + 1) * 512],
                            start=(k == 0),
                            stop=(k == 3),
                            perf_mode=DR,
                        )
                    copy_eng(
                        out=CS[0:1, g * 512 : (g + 1) * 512], in_=pc[0:1, :]
                    )

            for mi in range(MI):  # chunk A: block 0, ACT exp in place
                pg = sims(mi, 0, 1024)
                exp_act(pg, mi, 0, 1024)
            for mi in range(MI):  # chunk B: blocks 1-2a, fp8
                pg = sims(mi, 1024, 1536)
                exp_act(pg, mi, 1, 1536, fp8_off=0)
            SP = persist.tile([P, 2 * MI], f32)
            for mi in range(MI):  # chunks C (fp8, ACT) + D (block 4, DVE)
                pg = sims(mi, 2560, 1536)
                exp_act(pg, mi, 2, 1536, fp8_off=1536)
                sims_d_dve(mi)
            # own-row tail first: overlaps the mirror-colsum matmuls below
            DEXP = persist.tile([P, MI], f32)
            T1 = persist.tile([P, MI], f32)
            T2 = persist.tile([P, MI], f32)
            nc.scalar.activation(
                out=DEXP, in_=DIAG, func=AF.Exp, scale=INV_T, bias=bias_m2
            )
            nc.vector.reduce_sum(out=T1, in_=SPART, axis=AX.X)
            nc.vector.reduce_sum(out=T2, in_=SPARTV, axis=AX.X)
            nc.vector.tensor_add(T1, T1, T2)
            nc.vector.tensor_sub(SP[:, :MI], T1, DEXP)
            nc.vector.tensor_copy(out=SP[:, MI:], in_=POS)
            nc.sync.dma_start(out=sp_h[:, :], in_=SP)
            colsum([0, 1, 2], nc.vector.tensor_copy)
            colsum([3, 4, 5], nc.scalar.copy)
            nc.sync.dma_start(out=cs_h[:, :], in_=CS)

    nc.compile()
    _dedup_ldweights(nc, mybir)
    return nc


def get_nc():
    if "nc" not in _CACHE:
        _CACHE["nc"] = build_nc()
    return _CACHE["nc"]


def make_in_maps(zis: np.ndarray, zjs: np.ndarray):
    import ml_dtypes

    reps = np.concatenate(
        [np.asarray(zjs, np.float32), np.asarray(zis, np.float32)], axis=0
    )
    ones8 = np.ones((P, 2, 32), dtype=ml_dtypes.float8_e4m3)
    maps = []
    for c in range(NCORES):
        rot = np.roll(reps, -ROWS_PER_CORE * c, axis=0)[:NLOCAL]
        tiled = np.ascontiguousarray(
            rot.reshape(NT, P, D).transpose(1, 0, 2).reshape(P, NLOCAL)
        ).astype(ml_dtypes.bfloat16)
        maps.append({"reps": tiled, "ones8": ones8})
    return maps


def kernel(zis: np.ndarray, zjs: np.ndarray) -> np.ndarray:
    from concourse.bass_utils import run_bass_kernel_spmd

    nc = get_nc()
    in_maps = make_in_maps(zis, zjs)
    res = None
    for attempt in range(3):
        try:
            res = run_bass_kernel_spmd(nc, in_maps, core_ids=list(range(NCORES)))
            break
        except Exception:
            if attempt == 2:
                raise
            import time as _time

            _time.sleep(5.0)

    # host assembly: S[r] = own-row sum + mirror colsums; final ln in f64
    S = np.zeros(TWO_B, dtype=np.float64)
    pos_all = np.zeros(TWO_B, dtype=np.float64)
    for c, r in enumerate(res.results):
        i = np.arange(ROWS_PER_CORE)
        rows = (ROWS_PER_CORE * c + i) % TWO_B
        # sownpos layout [p, mi|mi+8] -> local row = 128*mi + p
        sp = np.asarray(r["sownpos"], np.float64)
        sown = sp[:, :MI].T.reshape(-1)
        pos = sp[:, MI:].T.reshape(-1)
        S[rows] += sown
        pos_all[rows] = pos
        cs = np.asarray(r["cs"], np.float64).reshape(-1)  # local cols 1024..4095
        gcols = (ROWS_PER_CORE * c + 1024 + np.arange(3072)) % TWO_B
        np.add.at(S, gcols, cs)
    loss = np.mean(-INV_T * pos_all + np.log(S) + SHIFT)
    return np.array(loss, dtype=np.float32)


# revision 32
# speedup vs baseline: 1.1459x; 1.1459x over previous
"""NT-Xent loss kernel for Trainium2, 8 NeuronCores.

Problem: B=4096 per view, D=128, temperature=0.1.
reps = concat([zjs, zis]) -> [8192, 128]; normalize rows; sim = normed @ normed.T;
loss = mean_i(-pos_i/T + logsumexp_{j!=i}(sim_ij/T)).

Symmetric-shard strategy (static SPMD, no collectives; host assembles):
  sim is symmetric, so each unordered pair needs computing once.  Core c gets
  reps rotated by -1024c rows (host-packed [p, t, d] bf16, 40 tiles = rows
  0..5119 of its rotated frame).  It computes the exp-block for its 1024 rows
  x 5120 cols (local column blocks 0..4 of 8):
    - blocks 0 (self-diagonal) and 4 (antipodal, recomputed by the partner
      core) contribute row sums only (ACT exp in place on PSUM / DVE
      Schraudolph-bf16 bit-trick exp, both with fused row accumulation),
    - blocks 1-3 are written as fp8e4 tiles; their row sums feed the local
      rows AND their column sums (fp8 DoubleRow ones-matmul over row pairs)
      are the mirror contributions to rows owned by cores c+1..c+3.  Cols
      5120..8191 of this core's rows arrive symmetrically as colsums from
      cores c-1..c-3.
  Everything on-device is in exp(10*s - 2) units (keeps fp8 in range; diag
  stays f32).  Outputs per core: own-row sums (diag-subtracted), positive-pair
  dots, 6x512 column sums.  The host assembles S per row across cores and
  takes the final ln in float64 (no on-device logs at all).

Scheduling notes: startup loads are split 2 tiles x 4 queues so the first
row-chunk's stats/transposes begin ~3us earlier; PE warm-up matmuls run
during the load phase to climb out of the low DVFS p-state; chunk order is
A | B | C/D interleaved per row-chunk (D's Schraudolph exps on DVE dovetail
with C's table exps on ACT); mirror colsums run after the next chunk's sims
so the PE never waits on an ACT->EXPB dependency.
"""

import numpy as np

B = 4096
D = 128
TWO_B = 2 * B
P = 128
NT = 40                   # loaded row tiles per core (rows 0..5119 local)
NLOCAL = NT * P           # 5120
MI = 8                    # own row-chunks (128 rows each -> 1024 rows)
NCORES = 8
ROWS_PER_CORE = TWO_B // NCORES  # 1024
INV_T = 10.0
SHIFT = 2.0               # on-device unit: exp(10 s - SHIFT)
LOG2E = 1.4426950408889634
SCHRAUDOLPH_C = 7.37      # calibrated: zero-mean bf16 bit-trick exp
A_DVE = float(np.float32(INV_T * 128.0 * LOG2E))
B_DVE = float(np.float32(128.0 * (127.0 - SHIFT * LOG2E) - SCHRAUDOLPH_C))

_CACHE = {}


def _dedup_ldweights(nc, mybir):
    """Drop InstLdweights that reload the identical stationary AP with no
    waits/updates: the PE array still holds those weights (bass emits one
    load per matmul; consecutive same-weight matmuls don't need it)."""
    removed = 0
    for fn in nc.m.functions:
        for blk in fn.blocks:
            insts = list(blk.instructions)
            new = []
            last_sig = None
            changed = False
            for i in insts:
                if isinstance(i, mybir.InstLdweights):
                    sig = str(i.ins[0]) + "|" + str(i.perf_mode)
                    if sig == last_sig and not i.has_wait() and not i.has_update():
                        removed += 1
                        changed = True
                        continue
                    last_sig = sig
                new.append(i)
            if changed:
                blk.instructions = new
    return removed


def build_nc():
    import concourse.bacc as bacc
    import concourse.bass as bass
    import concourse.mybir as mybir
    import concourse.tile as tile

    f32 = mybir.dt.float32
    bf16 = mybir.dt.bfloat16
    i16 = mybir.dt.int16
    fp8 = mybir.dt.float8e4
    AX = mybir.AxisListType
    OP = mybir.AluOpType
    AF = mybir.ActivationFunctionType
    DR = mybir.MatmulPerfMode.DoubleRow

    # Keep Ln+Exp in one activation table set (avoid mid-kernel reloads).
    from concourse import hw_specs

    _orig_tables = hw_specs.get_activation_tables

    def _patched_tables(arch):
        t = {k: set(v) for k, v in _orig_tables(arch).items()}
        for name, s in t.items():
            if name != "natural_log_exp_and_others":
                s.discard(AF.Exp)
                s.discard(AF.Ln)
        return t

    bacc.get_activation_tables = _patched_tables

    nc = bacc.Bacc(
        "TRN2",
        target_bir_lowering=False,
        debug=False,
        num_devices=NCORES,
    )
    # host-tiled: reps_h[p, t*128 + d] = bf16(reps_rot[t*128 + p, d]), t<40
    reps_h = nc.declare_dram_parameter("reps", [P, NLOCAL], bf16, isOutput=False)
    ones_h = nc.declare_dram_parameter("ones8", [P, 2, 32], fp8, isOutput=False)
    sp_h = nc.declare_dram_parameter("sownpos", [P, 2 * MI], f32, isOutput=True)
    cs_h = nc.declare_dram_parameter("cs", [1, 3072], f32, isOutput=True)

    with tile.TileContext(nc) as tc:
        with (
            tc.tile_pool(name="persist", bufs=1) as persist,
            tc.tile_pool(name="psum", bufs=2, space="PSUM") as psum,
            tc.tile_pool(name="scratch", bufs=2) as scratch,
        ):
            bias_m2 = persist.tile([P, 1], f32)
            nc.vector.memset(bias_m2, -SHIFT)
            ONES = persist.tile([P, 2, 32], fp8)
            nc.sync.dma_start(out=ONES, in_=ones_h[:, :, :])
            RAW = persist.tile([P, NT, P], bf16)
            SQ = persist.tile([P, NT, P], f32)
            HI = persist.tile([P, NT, P], bf16)
            HIT = persist.tile([P, NLOCAL], bf16)
            SS = persist.tile([P, NT], f32)
            LNSS = persist.tile([P, NT], f32)
            SCL = persist.tile([P, NT], f32)
            SPART = persist.tile([P, MI, 4], f32)
            SPARTV = persist.tile([P, MI, 2], f32)
            EXPB = persist.tile([P, MI, 3072], fp8)
            CS = persist.tile([1, 3072], f32)
            DIAG = persist.tile([P, MI], f32)
            POS = persist.tile([P, MI], f32)
            nc.vector.memset(SPART, 0.0)
            nc.vector.memset(SPARTV, 0.0)

            reps_t = reps_h[:, :].rearrange("p (t d) -> p t d", d=P)
            HIT3 = HIT.rearrange("d (t p) -> d t p", p=P)

            def scl_bcast(a, b):
                s = SCL[:, a:b]
                return bass.AP(
                    tensor=s.tensor, offset=s.offset, ap=list(s.ap) + [[0, P]]
                )

            def slice_stats(x, y):
                nc.vector.tensor_mul(SQ[:, x:y, :], RAW[:, x:y, :], RAW[:, x:y, :])
                nc.vector.reduce_sum(out=SS[:, x:y], in_=SQ[:, x:y, :], axis=AX.X)
                nc.scalar.activation(out=LNSS[:, x:y], in_=SS[:, x:y], func=AF.Ln)
                nc.scalar.activation(out=SCL[:, x:y], in_=LNSS[:, x:y],
                                     func=AF.Exp, scale=-0.5)

            def group0():
                # tiles 0..7 gate everything.  All stats pieces emit before
                # any HI-mul: an interleaved order serializes the pieces
                # through the DVE<->ACT rsqrt ping-pong (in-order queues).
                pieces = [(0, 3, nc.gpsimd), (3, 6, nc.sync), (6, 8, nc.scalar)]
                for x, y, ld in pieces:
                    ld.dma_start(out=RAW[:, x:y, :], in_=reps_t[:, x:y, :])
                for x, y, _ in pieces:
                    slice_stats(x, y)
                for (x, y, _), tr in zip(pieces, [nc.sync, nc.scalar, nc.sync]):
                    nc.vector.tensor_mul(
                        HI[:, x:y, :], RAW[:, x:y, :], scl_bcast(x, y)
                    )
                    tr.dma_start_transpose(
                        out=HIT3[:, x:y, :], in_=HI[:, x:y, :]
                    )

            def group(a, b, sq_eng=None):
                mid = a + (b - a) // 2
                splits = [(a, mid, nc.gpsimd), (mid, b, nc.scalar)]
                for x, y, eng in splits:
                    eng.dma_start(out=RAW[:, x:y, :], in_=reps_t[:, x:y, :])
                sq = sq_eng.tensor_mul if sq_eng else nc.vector.tensor_mul
                for x, y, _ in splits:
                    sq(SQ[:, x:y, :], RAW[:, x:y, :], RAW[:, x:y, :])
                    nc.vector.reduce_sum(
                        out=SS[:, x:y], in_=SQ[:, x:y, :], axis=AX.X
                    )
                nc.scalar.activation(out=LNSS[:, a:b], in_=SS[:, a:b], func=AF.Ln)
                nc.scalar.activation(
                    out=SCL[:, a:b], in_=LNSS[:, a:b], func=AF.Exp, scale=-0.5
                )
                for x in range(a, b, 4):
                    nc.vector.tensor_mul(
                        HI[:, x : x + 4, :],
                        RAW[:, x : x + 4, :],
                        scl_bcast(x, x + 4),
                    )
                    nc.sync.dma_start_transpose(
                        out=HIT3[:, x : x + 4, :], in_=HI[:, x : x + 4, :]
                    )

            group0()
            # diag_i = ||hi_i||^2 with the same bf16 values the matmul uses
            for mi in range(MI):
                jd = scratch.tile([P, P], f32, tag="ttr_junk")
                nc.vector.scalar_tensor_tensor(
                    out=jd,
                    in0=HI[:, mi, :],
                    scalar=1.0,
                    in1=HI[:, mi, :],
                    op0=OP.mult,
                    op1=OP.mult,
                    accum_out=DIAG[:, mi : mi + 1],
                )
            group(8, 20)
            group(20, 32)
            group(32, 40)
            # positive-pair dots: rows mi pair with antipodal tile 32+mi
            for mi in range(MI):
                jp = scratch.tile([P, P], f32, tag="ttr_junk")
                nc.vector.scalar_tensor_tensor(
                    out=jp,
                    in0=HI[:, mi, :],
                    scalar=1.0,
                    in1=HI[:, NT - MI + mi, :],
                    op0=OP.mult,
                    op1=OP.mult,
                    accum_out=POS[:, mi : mi + 1],
                )

            # ---------------- Phase B ----------------------------------------
            def sims(mi, c0, ncols):
                lhsT = HIT[:, mi * P : (mi + 1) * P]
                pg = psum.tile([P, 1536], f32, tag="pg")
                for k in range(ncols // 512):
                    nc.tensor.matmul(
                        pg[:, k * 512 : (k + 1) * 512],
                        lhsT,
                        HIT[:, c0 + k * 512 : c0 + (k + 1) * 512],
                        start=True,
                        stop=True,
                    )
                return pg

            def exp_act(pg, mi, ci, ncols, fp8_off=None):
                out = (
                    pg[:, :ncols]
                    if fp8_off is None
                    else EXPB[:, mi, fp8_off : fp8_off + ncols]
                )
                nc.scalar.activation(
                    out=out,
                    in_=pg[:, :ncols],
                    func=AF.Exp,
                    scale=INV_T,
                    bias=bias_m2,
                    accum_out=SPART[:, mi, ci : ci + 1],
                )

            def exp_dve_half(pgd, mi, h):
                jq = scratch.tile([P, 512], i16, tag="jq")
                nc.vector.tensor_scalar(
                    out=jq,
                    in0=pgd[:, :],
                    scalar1=A_DVE,
                    scalar2=B_DVE,
                    op0=OP.mult,
                    op1=OP.add,
                )
                nc.vector.reduce_sum(
                    out=SPARTV[:, mi, h : h + 1], in_=jq.bitcast(bf16), axis=AX.X
                )

            def sims_d_dve(mi):
                # block 4 on its own 1-bank psum ring: decouples the DVE
                # consumer from ACT's ring so the PE never waits on both
                lhsT = HIT[:, mi * P : (mi + 1) * P]
                for h in range(2):
                    pgd = psum.tile([P, 512], f32, tag="cs")
                    nc.tensor.matmul(
                        pgd[:, :],
                        lhsT,
                        HIT[:, 4096 + h * 512 : 4096 + (h + 1) * 512],
                        start=True,
                        stop=True,
                    )
                    exp_dve_half(pgd, mi, h)

            def colsum(groups, copy_eng):
                # mirror colsums over this core's 1024 rows: fp8 DoubleRow
                # ones-matmul, 2 row-tiles per pass, accumulate 4 passes
                for g in groups:
                    pc = psum.tile([P, 512], f32, tag="cs")
                    for k in range(4):
                        nc.tensor.matmul(
                            pc[0:32, :],
                            ONES[:, :, :],
                            EXPB[:, 2 * k : 2 * k + 2, g * 512 : (g + 1) * 512],
                            start=(k == 0),
                            stop=(k == 3),
                            perf_mode=DR,
                        )
                    copy_eng(
                        out=CS[0:1, g * 512 : (g + 1) * 512], in_=pc[0:1, :]
                    )

            for mi in range(MI):  # chunk A: block 0, ACT exp in place
                pg = sims(mi, 0, 1024)
                exp_act(pg, mi, 0, 1024)
            for mi in range(MI):  # chunk B: blocks 1-2a, fp8
                pg = sims(mi, 1024, 1536)
                exp_act(pg, mi, 1, 1536, fp8_off=0)
            SP = persist.tile([P, 2 * MI], f32)
            for mi in range(MI):  # chunks C (fp8, ACT) + D (block 4, DVE)
                pg = sims(mi, 2560, 1536)
                exp_act(pg, mi, 2, 1536, fp8_off=1536)
                sims_d_dve(mi)
            # own-row tail first: overlaps the mirror-colsum matmuls below
            DEXP = persist.tile([P, MI], f32)
            T1 = persist.tile([P, MI], f32)
            T2 = persist.tile([P, MI], f32)
            nc.scalar.activation(
                out=DEXP, in_=DIAG, func=AF.Exp, scale=INV_T, bias=bias_m2
            )
            nc.vector.reduce_sum(out=T1, in_=SPART, axis=AX.X)
            nc.vector.reduce_sum(out=T2, in_=SPARTV, axis=AX.X)
            nc.vector.tensor_add(T1, T1, T2)
            nc.vector.tensor_sub(SP[:, :MI], T1, DEXP)
            nc.vector.tensor_copy(out=SP[:, MI:], in_=POS)
            nc.sync.dma_start(out=sp_h[:, :], in_=SP)
            colsum([0, 1, 2], nc.vector.tensor_copy)
            colsum([3, 4, 5], nc.scalar.copy)
            nc.sync.dma_start(out=cs_h[:, :], in_=CS)

    nc.compile()
    _dedup_ldweights(nc, mybir)
    return nc


def get_nc():
    if "nc" not in _CACHE:
        _CACHE["nc"] = build_nc()
    return _CACHE["nc"]


def make_in_maps(zis: np.ndarray, zjs: np.ndarray):
    import ml_dtypes

    reps = np.concatenate(
        [np.asarray(zjs, np.float32), np.asarray(zis, np.float32)], axis=0
    )
    ones8 = np.ones((P, 2, 32), dtype=ml_dtypes.float8_e4m3)
    maps = []
    for c in range(NCORES):
        rot = np.roll(reps, -ROWS_PER_CORE * c, axis=0)[:NLOCAL]
        tiled = np.ascontiguousarray(
            rot.reshape(NT, P, D).transpose(1, 0, 2).reshape(P, NLOCAL)
        ).astype(ml_dtypes.bfloat16)
        maps.append({"reps": tiled, "ones8": ones8})
    return maps


def kernel(zis: np.ndarray, zjs: np.ndarray) -> np.ndarray:
    from concourse.bass_utils import run_bass_kernel_spmd

    nc = get_nc()
    in_maps = make_in_maps(zis, zjs)
    res = None
    for attempt in range(3):
        try:
            res = run_bass_kernel_spmd(nc, in_maps, core_ids=list(range(NCORES)))
            break
        except Exception:
            if attempt == 2:
                raise
            import time as _time

            _time.sleep(5.0)

    # host assembly: S[r] = own-row sum + mirror colsums; final ln in f64
    S = np.zeros(TWO_B, dtype=np.float64)
    pos_all = np.zeros(TWO_B, dtype=np.float64)
    for c, r in enumerate(res.results):
        i = np.arange(ROWS_PER_CORE)
        rows = (ROWS_PER_CORE * c + i) % TWO_B
        # sownpos layout [p, mi|mi+8] -> local row = 128*mi + p
        sp = np.asarray(r["sownpos"], np.float64)
        sown = sp[:, :MI].T.reshape(-1)
        pos = sp[:, MI:].T.reshape(-1)
        S[rows] += sown
        pos_all[rows] = pos
        cs = np.asarray(r["cs"], np.float64).reshape(-1)  # local cols 1024..4095
        gcols = (ROWS_PER_CORE * c + 1024 + np.arange(3072)) % TWO_B
        np.add.at(S, gcols, cs)
    loss = np.mean(-INV_T * pos_all + np.log(S) + SHIFT)
    return np.array(loss, dtype=np.float32)
